# revision 7
# baseline (speedup 1.0000x reference)
"""Trainium2 Bass kernel for nn_PiPoolLayer (segment_reduce).

Computation (see reference): per bond e with feature x_e [6*128]:
    feat_e = relu(x_e @ W1 + b1)                    # [128]
    pooled[g,t] = sum_{e in (graph g, type t)} feat_e
    logits[g,t] = pooled[g,t] . w2 + b2, masked where count==0
    out[g] = softmax(logits[g])                     # [B=128, 36]

Sharding: data-parallel over graphs — 16 graphs per core on 8 cores; the
segment reduction is fully local per core, no cross-core communication.

Per-core dataflow (bonds processed in groups of 512):
  DMA   : bond_feat rows [512, 768] -> SBUF [128, 4*768]
  PE    : 6x4 transposes (identity matmul) -> bondT chunks [feat128, bond]
  DVE   : PSUM -> SBUF copies of the transposed chunks
  PE    : 6 accumulating matmuls lhsT=W1_chunk [128,128] (stationary),
          rhs=bondT chunk [128, 512] streaming, float32r fast path
          -> featT PSUM [hidden=128, bond=512]
  ACT   : per segment run: relu(featT + b1) with accum_out
          -> pooled[:, seg] (free-dim segment sum fused with bias+relu)
  PE    : logits = w2 . pooled  (contract hidden over partitions)
  DVE/ACT: +b2, count mask, numerically-stable softmax over 36 types
"""

import os
import sys

if "/opt/trn_rl_repo" not in sys.path:
    sys.path.insert(0, "/opt/trn_rl_repo")

import numpy as np
from contextlib import ExitStack

import concourse.bass as bass
import concourse.tile as tile
from concourse import bacc, masks, mybir
from concourse.bass_utils import run_bass_kernel_spmd

N_CORES = 8
NUM_TYPE = 36
HIDDEN = 128
FEAT = 768          # 6 angles * 128 bond_dim
NCHUNK = 6          # FEAT / 128
GROUP = 512         # bonds per pipeline group (one PSUM bank of featT)
NEG_INF = -1e9

# matmul dtype for the bond_feat @ W1 contraction ('f32r' | 'f32' | 'bf16')
MM_MODE = os.environ.get("PIPOOL_MM", "f32r")
# dtype for the PE transposes ('f32r' | 'f32')
TR_MODE = os.environ.get("PIPOOL_TR", "f32r")

_CACHE = {}


# ---------------------------------------------------------------------------
# host-side structure derivation
# ---------------------------------------------------------------------------

def _derive_plan(seg_counts, e_core):
    """Per-group ACT piece plan from per-core segment lengths.

    Returns (groups, pieces, n_bscratch) where pieces[i] is the list for
    group i of (off_in_group, length, seg_idx, bscratch_slot_or_None).
    """
    n_seg = len(seg_counts)
    seg_ends = np.cumsum(seg_counts)
    seg_starts = seg_ends - seg_counts
    groups = [(s, min(GROUP, e_core - s)) for s in range(0, e_core, GROUP)]
    n_pieces = np.zeros(n_seg, np.int32)
    for g0, sz in groups:
        g1 = g0 + sz
        s_lo = int(np.searchsorted(seg_ends, g0, side="right"))
        s_hi = int(np.searchsorted(seg_starts, g1, side="left"))
        for s in range(s_lo, s_hi):
            if seg_counts[s] > 0:
                n_pieces[s] += 1
    bslot = {}
    nb = 0
    pieces = []
    for g0, sz in groups:
        g1 = g0 + sz
        plist = []
        s_lo = int(np.searchsorted(seg_ends, g0, side="right"))
        s_hi = int(np.searchsorted(seg_starts, g1, side="left"))
        for s in range(s_lo, s_hi):
            if seg_counts[s] == 0:
                continue
            a = max(int(seg_starts[s]), g0)
            b = min(int(seg_ends[s]), g1)
            if n_pieces[s] == 1:
                plist.append((a - g0, b - a, s, None))
            else:
                slot = nb
                nb += 1
                bslot.setdefault(s, []).append(slot)
                plist.append((a - g0, b - a, s, slot))
        pieces.append(plist)
    return groups, pieces, nb, bslot


def _build_program(e_core, g_core, seg_counts):
    n_seg = len(seg_counts)
    groups, pieces, n_bscratch, bslot = _derive_plan(seg_counts, e_core)

    f32 = mybir.dt.float32
    f32r = mybir.dt.float32r
    bf16 = mybir.dt.bfloat16

    nc = bacc.Bacc("TRN2", target_bir_lowering=False, debug=False,
                   num_devices=N_CORES)
    bf = nc.dram_tensor("bf", [e_core, FEAT], f32, kind="ExternalInput").ap()
    w1 = nc.dram_tensor("w1", [FEAT, HIDDEN], f32, kind="ExternalInput").ap()
    b1 = nc.dram_tensor("b1", [HIDDEN], f32, kind="ExternalInput").ap()
    w2 = nc.dram_tensor("w2", [HIDDEN], f32, kind="ExternalInput").ap()
    bvec = nc.dram_tensor("bvec", [2], f32, kind="ExternalInput").ap()  # [b2, 0]
    mmul = nc.dram_tensor("mmul", [g_core, NUM_TYPE], f32, kind="ExternalInput").ap()
    madd = nc.dram_tensor("madd", [g_core, NUM_TYPE], f32, kind="ExternalInput").ap()
    y = nc.dram_tensor("y", [g_core, NUM_TYPE], f32, kind="ExternalOutput").ap()

    if MM_MODE == "bf16":
        mm_dt = bf16
    elif MM_MODE == "f32":
        mm_dt = f32
    else:
        mm_dt = f32r

    with tile.TileContext(nc) as tc, ExitStack() as ctx:
        const_pool = ctx.enter_context(tc.tile_pool(name="const", bufs=1))
        ld_pool = ctx.enter_context(tc.tile_pool(name="ld", bufs=4))
        bt_pool = ctx.enter_context(tc.tile_pool(name="bt", bufs=3))
        sc_pool = ctx.enter_context(tc.tile_pool(name="sc", bufs=2))
        tp_pool = ctx.enter_context(
            tc.tile_pool(name="tp", bufs=4, space="PSUM"))
        ft_pool = ctx.enter_context(
            tc.tile_pool(name="ft", bufs=3, space="PSUM"))

        ident_f32 = const_pool.tile([128, 128], f32)
        masks.make_identity(nc, ident_f32[:])
        ident = ident_f32[:]

        w1_sb = const_pool.tile([128, NCHUNK * HIDDEN], f32)
        nc.sync.dma_start(
            w1_sb[:], w1.rearrange("(c p) h -> p c h", p=128))
        if mm_dt != f32:
            w1_mm = const_pool.tile([128, NCHUNK * HIDDEN], mm_dt)
            nc.vector.tensor_copy(w1_mm[:], w1_sb[:])
        else:
            w1_mm = w1_sb

        b1_sb = const_pool.tile([128, 1], f32)
        nc.sync.dma_start(b1_sb[:], b1.rearrange("(p o) -> p o", o=1))
        w2_sb = const_pool.tile([128, 1], f32)
        nc.sync.dma_start(w2_sb[:], w2.rearrange("(p o) -> p o", o=1))
        bvec_sb = const_pool.tile([1, 2], f32)
        nc.sync.dma_start(bvec_sb[:], bvec.rearrange("(p o) -> p o", p=1))
        mmul_sb = const_pool.tile([g_core, NUM_TYPE], f32)
        nc.sync.dma_start(mmul_sb[:], mmul)
        madd_sb = const_pool.tile([g_core, NUM_TYPE], f32)
        nc.sync.dma_start(madd_sb[:], madd)

        pooled = const_pool.tile([128, n_seg], f32)
        nc.vector.memset(pooled[:], 0.0)
        if n_bscratch:
            bsc = const_pool.tile([128, n_bscratch], f32)

        for gi, (g0, sz) in enumerate(groups):
            nsub = (sz + 127) // 128
            # ---- load [sz, FEAT] as [p, sub, FEAT] ----
            if sz % 128 == 0:
                ldt = ld_pool.tile([128, nsub * FEAT], f32, tag="ld")
                nc.sync.dma_start(
                    ldt[:].rearrange("p (s f) -> p s f", s=nsub),
                    bf[g0:g0 + sz, :].rearrange("(s p) f -> p s f", p=128))
                subs = [128] * nsub
            else:
                assert nsub == 1
                ldt = ld_pool.tile([sz, FEAT], f32, tag="ldtail")
                nc.sync.dma_start(ldt[:], bf[g0:g0 + sz, :])
                subs = [sz]

            featT = ft_pool.tile([128, sz], f32, tag="ft")
            for c in range(NCHUNK):
                tp = tp_pool.tile([128, sz], f32, tag="tp")
                for s, ssz in enumerate(subs):
                    src = ldt[0:ssz, s * FEAT + c * 128:
                              s * FEAT + c * 128 + 128]
                    dst = tp[:, s * 128: s * 128 + ssz]
                    nc.tensor.transpose(dst, src, ident[0:ssz, 0:ssz])
                btc = bt_pool.tile([128, sz], mm_dt, tag=f"bt{c}")
                nc.vector.tensor_copy(btc[:], tp[:])
                nc.tensor.matmul(
                    featT[:],
                    w1_mm[:, c * HIDDEN:(c + 1) * HIDDEN],
                    btc[:],
                    start=(c == 0), stop=(c == NCHUNK - 1))

            # ---- segment pooling: relu(featT + b1), summed over free dim ----
            for (off, ln, s, slot) in pieces[gi]:
                scr = sc_pool.tile([128, GROUP], f32, tag="scr")
                tgt = pooled[:, s:s + 1] if slot is None else bsc[:, slot:slot + 1]
                nc.scalar.activation(
                    scr[:, 0:ln], featT[:, off:off + ln],
                    mybir.ActivationFunctionType.Relu,
                    bias=b1_sb[:, 0:1], scale=1.0,
                    accum_out=tgt)

        # combine split-segment partial sums
        for s, slots in bslot.items():
            nc.vector.tensor_add(
                pooled[:, s:s + 1], bsc[:, slots[0]:slots[0] + 1],
                bsc[:, slots[1]:slots[1] + 1])
            for extra in slots[2:]:
                nc.vector.tensor_add(
                    pooled[:, s:s + 1], pooled[:, s:s + 1],
                    bsc[:, extra:extra + 1])

        # ---- logits = w2 . pooled (+b2), mask, softmax over types ----
        smf = const_pool.tile([1, n_seg], f32)
        for o in range(0, n_seg, 512):
            n = min(512, n_seg - o)
            lg = ft_pool.tile([1, n], f32, tag="lg", bufs=1)
            nc.tensor.matmul(lg[:], w2_sb[:], pooled[:, o:o + n],
                             start=True, stop=True)
            nc.scalar.activation(
                smf[:, o:o + n], lg[:],
                mybir.ActivationFunctionType.Identity,
                bias=bvec_sb[0:1, 0:1], scale=1.0)

        smg = const_pool.tile([g_core, NUM_TYPE], f32)
        nc.sync.dma_start(smg[:], smf[:])
        # masked = logits * mmul + madd   (mmul in {1,0}, madd in {0,-1e9})
        msk = const_pool.tile([g_core, NUM_TYPE], f32)
        nc.vector.tensor_mul(msk[:], smg[:], mmul_sb[:])
        nc.vector.tensor_add(msk[:], msk[:], madd_sb[:])

        mx = const_pool.tile([g_core, 1], f32)
        nc.vector.reduce_max(mx[:], msk[:], axis=mybir.AxisListType.X)
        nmx = const_pool.tile([g_core, 1], f32)
        nc.vector.tensor_scalar_mul(nmx[:], mx[:], -1.0)
        ex = const_pool.tile([g_core, NUM_TYPE], f32)
        sume = const_pool.tile([g_core, 1], f32)
        nc.scalar.activation(ex[:], msk[:],
                             mybir.ActivationFunctionType.Exp,
                             bias=nmx[:, 0:1], scale=1.0,
                             accum_out=sume[:])
        rse = const_pool.tile([g_core, 1], f32)
        nc.vector.reciprocal(rse[:], sume[:])
        pr = const_pool.tile([g_core, NUM_TYPE], f32)
        nc.vector.tensor_scalar_mul(pr[:], ex[:], rse[:, 0:1])
        nc.sync.dma_start(y, pr[:])

    nc.compile()
    return nc


# ---------------------------------------------------------------------------
# host wrapper
# ---------------------------------------------------------------------------

def _numpy_reference(bond_types_batch, type_count_batch, bond_feat,
                     W1, b1, w2, b2):
    counts = type_count_batch.reshape(-1, NUM_TYPE)
    Bn = counts.shape[0]
    E_ = bond_feat.shape[0]
    feat = np.maximum(bond_feat.reshape(E_, -1) @ W1 + b1, 0.0)
    bonds_per_graph = counts.sum(axis=1)
    cum = np.cumsum(bonds_per_graph)
    graph_id = np.searchsorted(cum, np.arange(E_), side="right")
    seg = graph_id * NUM_TYPE + bond_types_batch
    pooled = np.zeros((Bn * NUM_TYPE, feat.shape[1]), np.float32)
    np.add.at(pooled, seg, feat)
    logits = (pooled @ w2 + b2[0]).reshape(Bn, NUM_TYPE)
    logits = np.where(counts > 0, logits, np.float32(NEG_INF))
    m = logits.max(axis=1, keepdims=True)
    e = np.exp(logits - m)
    return (e / e.sum(axis=1, keepdims=True)).astype(np.float32)


def kernel(**inputs):
    bond_types = np.asarray(inputs["bond_types_batch"])
    counts = np.asarray(inputs["type_count_batch"]).astype(np.int64)
    bond_feat = np.ascontiguousarray(np.asarray(inputs["bond_feat"],
                                                dtype=np.float32))
    W1 = np.asarray(inputs["W1"], dtype=np.float32)
    b1 = np.asarray(inputs["b1"], dtype=np.float32)
    w2 = np.asarray(inputs["w2"], dtype=np.float32)
    b2 = np.asarray(inputs["b2"], dtype=np.float32)

    n_graph = counts.size // NUM_TYPE
    e_total = bond_feat.shape[0]
    bf2d = bond_feat.reshape(e_total, -1)

    ok = (
        n_graph % N_CORES == 0
        and bf2d.shape[1] == FEAT
        and counts.sum() == e_total
        and np.array_equal(
            np.repeat(np.tile(np.arange(NUM_TYPE), n_graph), counts),
            bond_types)
    )
    if ok:
        # identical per-core structure required for a single SPMD program
        per_core = counts.reshape(N_CORES, -1)
        ok = bool((per_core == per_core[0]).all())
    if not ok:
        return _numpy_reference(bond_types, counts.astype(np.int32),
                                bond_feat, W1, b1, w2, b2)

    g_core = n_graph // N_CORES
    seg_counts = per_core[0]
    e_core = int(seg_counts.sum())

    key = (e_core, g_core, seg_counts.tobytes(), MM_MODE, TR_MODE)
    if key not in _CACHE:
        _CACHE[key] = _build_program(e_core, g_core, seg_counts)
    nc = _CACHE[key]

    counts2d = counts.reshape(n_graph, NUM_TYPE)
    mmul_full = (counts2d > 0).astype(np.float32)
    madd_full = np.where(counts2d > 0, 0.0, NEG_INF).astype(np.float32)
    bvec = np.array([b2[0], 0.0], np.float32)

    in_maps = []
    for c in range(N_CORES):
        in_maps.append({
            "bf": bf2d[c * e_core:(c + 1) * e_core],
            "w1": W1, "b1": b1, "w2": w2, "bvec": bvec,
            "mmul": mmul_full[c * g_core:(c + 1) * g_core],
            "madd": madd_full[c * g_core:(c + 1) * g_core],
        })
    res = run_bass_kernel_spmd(nc, in_maps, core_ids=list(range(N_CORES)))
    return np.concatenate([res.results[c]["y"] for c in range(N_CORES)], axis=0)


# revision 15
# speedup vs baseline: 1.1709x; 1.1709x over previous
"""Trainium2 Bass kernel for nn_PiPoolLayer (segment_reduce).

Computation (see reference): per bond e with feature x_e [6*128]:
    feat_e = relu(x_e @ W1 + b1)                    # [128]
    pooled[g,t] = sum_{e in (graph g, type t)} feat_e
    logits[g,t] = pooled[g,t] . w2 + b2, masked where count==0
    out[g] = softmax(logits[g])                     # [B=128, 36]

Sharding: data-parallel over graphs — 16 graphs per core on 8 cores; the
segment reduction is fully local per core, no cross-core communication.

Per-core dataflow (bonds processed in groups of 512):
  DMA   : bond_feat rows [512, 768] -> SBUF [128, 4*768]
  PE    : 6x4 transposes (identity matmul) -> bondT chunks [feat128, bond]
  DVE   : PSUM -> SBUF copies of the transposed chunks
  PE    : 6 accumulating matmuls lhsT=W1_chunk [128,128] (stationary),
          rhs=bondT chunk [128, 512] streaming, float32r fast path
          -> featT PSUM [hidden=128, bond=512]
  ACT   : per segment run: relu(featT + b1) with accum_out
          -> pooled[:, seg] (free-dim segment sum fused with bias+relu)
  PE    : logits = w2 . pooled  (contract hidden over partitions)
  DVE/ACT: +b2, count mask, numerically-stable softmax over 36 types
"""

import os
import sys

if "/opt/trn_rl_repo" not in sys.path:
    sys.path.insert(0, "/opt/trn_rl_repo")

import numpy as np
from contextlib import ExitStack

import concourse.bass as bass
import concourse.tile as tile
from concourse import bacc, masks, mybir
from concourse.bass_utils import run_bass_kernel_spmd

N_CORES = 8
NUM_TYPE = 36
HIDDEN = 128
FEAT = 768          # 6 angles * 128 bond_dim
NCHUNK = 6          # FEAT / 128
GROUP = 512         # bonds per pipeline group (one PSUM bank of featT)
NEG_INF = -1e9

# matmul dtype for the bond_feat @ W1 contraction ('f32r' | 'f32' | 'bf16')
MM_MODE = os.environ.get("PIPOOL_MM", "f32r")
# dtype for the PE transposes ('f32r' | 'f32')
TR_MODE = os.environ.get("PIPOOL_TR", "f32r")

_CACHE = {}


# ---------------------------------------------------------------------------
# host-side structure derivation
# ---------------------------------------------------------------------------

def _derive_plan(seg_counts, e_core):
    """Per-group ACT piece plan from per-core segment lengths.

    Returns (groups, pieces, n_bscratch) where pieces[i] is the list for
    group i of (off_in_group, length, seg_idx, bscratch_slot_or_None).
    """
    n_seg = len(seg_counts)
    seg_ends = np.cumsum(seg_counts)
    seg_starts = seg_ends - seg_counts
    groups = [(s, min(GROUP, e_core - s)) for s in range(0, e_core, GROUP)]
    n_pieces = np.zeros(n_seg, np.int32)
    for g0, sz in groups:
        g1 = g0 + sz
        s_lo = int(np.searchsorted(seg_ends, g0, side="right"))
        s_hi = int(np.searchsorted(seg_starts, g1, side="left"))
        for s in range(s_lo, s_hi):
            if seg_counts[s] > 0:
                n_pieces[s] += 1
    bslot = {}
    nb = 0
    pieces = []
    for g0, sz in groups:
        g1 = g0 + sz
        plist = []
        s_lo = int(np.searchsorted(seg_ends, g0, side="right"))
        s_hi = int(np.searchsorted(seg_starts, g1, side="left"))
        for s in range(s_lo, s_hi):
            if seg_counts[s] == 0:
                continue
            a = max(int(seg_starts[s]), g0)
            b = min(int(seg_ends[s]), g1)
            if n_pieces[s] == 1:
                plist.append((a - g0, b - a, s, None))
            else:
                slot = nb
                nb += 1
                bslot.setdefault(s, []).append(slot)
                plist.append((a - g0, b - a, s, slot))
        pieces.append(plist)
    return groups, pieces, nb, bslot


def _build_program(e_core, g_core, seg_counts):
    n_seg = len(seg_counts)
    groups, pieces, n_bscratch, bslot = _derive_plan(seg_counts, e_core)

    f32 = mybir.dt.float32
    f32r = mybir.dt.float32r
    bf16 = mybir.dt.bfloat16

    nc = bacc.Bacc("TRN2", target_bir_lowering=False, debug=False,
                   num_devices=N_CORES)
    bf = nc.dram_tensor("bf", [e_core, FEAT], f32, kind="ExternalInput").ap()
    w1 = nc.dram_tensor("w1", [FEAT, HIDDEN], f32, kind="ExternalInput").ap()
    b1 = nc.dram_tensor("b1", [HIDDEN], f32, kind="ExternalInput").ap()
    w2 = nc.dram_tensor("w2", [HIDDEN], f32, kind="ExternalInput").ap()
    bvec = nc.dram_tensor("bvec", [2], f32, kind="ExternalInput").ap()  # [b2, 0]
    idin = nc.dram_tensor("idin", [128, 128], f32, kind="ExternalInput").ap()
    mmul = nc.dram_tensor("mmul", [g_core, NUM_TYPE], f32, kind="ExternalInput").ap()
    madd = nc.dram_tensor("madd", [g_core, NUM_TYPE], f32, kind="ExternalInput").ap()
    y = nc.dram_tensor("y", [g_core, NUM_TYPE], f32, kind="ExternalOutput").ap()

    if MM_MODE == "bf16":
        mm_dt = bf16
    elif MM_MODE == "f32":
        mm_dt = f32
    else:
        mm_dt = f32r

    with tile.TileContext(nc) as tc, ExitStack() as ctx:
        const_pool = ctx.enter_context(tc.tile_pool(name="const", bufs=1))
        ld_pool = ctx.enter_context(tc.tile_pool(name="ld", bufs=8))
        bt_pool = ctx.enter_context(tc.tile_pool(name="bt", bufs=3))
        sc_pool = ctx.enter_context(tc.tile_pool(name="sc", bufs=2))
        rw_pool = ctx.enter_context(tc.tile_pool(name="rw", bufs=2))
        tp_pool = ctx.enter_context(
            tc.tile_pool(name="tp", bufs=3, space="PSUM"))
        ft_pool = ctx.enter_context(
            tc.tile_pool(name="ft", bufs=2, space="PSUM"))

        ident_f32 = const_pool.tile([128, 128], f32)
        nc.scalar.dma_start(ident_f32[:], idin)
        ident = ident_f32[:]

        w1_sb = const_pool.tile([128, NCHUNK * HIDDEN], f32)
        nc.scalar.dma_start(
            w1_sb[:], w1.rearrange("(c p) h -> p c h", p=128))
        if mm_dt != f32:
            w1_mm = const_pool.tile([128, NCHUNK * HIDDEN], mm_dt)
            nc.vector.tensor_copy(w1_mm[:], w1_sb[:])
        else:
            w1_mm = w1_sb

        b1_sb = const_pool.tile([128, 1], f32)
        nc.scalar.dma_start(b1_sb[:], b1.rearrange("(p o) -> p o", o=1))
        w2_sb = const_pool.tile([128, 1], f32)
        nc.scalar.dma_start(w2_sb[:], w2.rearrange("(p o) -> p o", o=1))
        bvec_sb = const_pool.tile([1, 2], f32)
        nc.scalar.dma_start(bvec_sb[:], bvec.rearrange("(p o) -> p o", p=1))
        mmul_sb = const_pool.tile([g_core, NUM_TYPE], f32)
        nc.scalar.dma_start(mmul_sb[:], mmul)
        madd_sb = const_pool.tile([g_core, NUM_TYPE], f32)
        nc.scalar.dma_start(madd_sb[:], madd)

        pooled = const_pool.tile([128, n_seg], f32)
        nc.vector.memset(pooled[:], 0.0)
        if n_bscratch:
            bsc = const_pool.tile([128, n_bscratch], f32)

        # Uniform segment length -> pool via one ACT relu per group into a
        # 57-aligned SBUF window, then one wide DVE tensor_reduce per window.
        uniform = bool((seg_counts == seg_counts[0]).all()) and seg_counts[0] > 0
        per = int(seg_counts[0]) if uniform else 0
        if uniform:
            K_w = max(1, 2280 // per)
            W = K_w * per
            rw_tiles = {}

        def win_of(b):  # window index containing bond b
            return b // W

        copy_engines = [nc.vector, nc.scalar]

        for gi, (g0, sz) in enumerate(groups):
            nsub = (sz + 127) // 128
            # ---- load [sz, FEAT] as [p, sub, FEAT] ----
            if sz % 128 == 0:
                ldt = ld_pool.tile([128, nsub * FEAT], f32, tag="ld")
                nc.sync.dma_start(
                    ldt[:].rearrange("p (s f) -> p s f", s=nsub),
                    bf[g0:g0 + sz, :].rearrange("(s p) f -> p s f", p=128))
                subs = [128] * nsub
            else:
                assert nsub == 1
                ldt = ld_pool.tile([sz, FEAT], f32, tag="ldtail")
                nc.sync.dma_start(ldt[:], bf[g0:g0 + sz, :])
                subs = [sz]

            featT = ft_pool.tile([128, sz], f32, tag="ft")
            btps = []
            for pr_i in range(NCHUNK // 2):
                tp = tp_pool.tile([128, 2 * sz], f32, tag="tp")
                for half in range(2):
                    c = 2 * pr_i + half
                    for s, ssz in enumerate(subs):
                        src = ldt[0:ssz, s * FEAT + c * 128:
                                  s * FEAT + c * 128 + 128]
                        dst = tp[:, half * sz + s * 128:
                                 half * sz + s * 128 + ssz]
                        nc.tensor.transpose(dst, src, ident[0:ssz, 0:ssz])
                btp = bt_pool.tile([128, 2 * sz], mm_dt, tag=f"bt{pr_i}")
                eng = copy_engines[(gi * 3 + pr_i) % 2]
                if eng is nc.scalar:
                    nc.scalar.copy(btp[:], tp[:])
                else:
                    nc.vector.tensor_copy(btp[:], tp[:])
                btps.append(btp)
            for c in range(NCHUNK):
                nc.tensor.matmul(
                    featT[:],
                    w1_mm[:, c * HIDDEN:(c + 1) * HIDDEN],
                    btps[c // 2][:, (c % 2) * sz:(c % 2 + 1) * sz],
                    start=(c == 0), stop=(c == NCHUNK - 1))

            # ---- relu(featT + b1) -> window buffers / segment pooling ----
            if uniform:
                g1 = g0 + sz
                for w in range(win_of(g0), win_of(g1 - 1) + 1):
                    ws = w * W
                    we = min(ws + W, e_core)
                    if w not in rw_tiles:
                        rw_tiles[w] = rw_pool.tile([128, we - ws], f32,
                                                   tag="rw", name=f"rw{w}")
                    a, b = max(g0, ws), min(g1, we)
                    nc.scalar.activation(
                        rw_tiles[w][:, a - ws:b - ws],
                        featT[:, a - g0:b - g0],
                        mybir.ActivationFunctionType.Relu,
                        bias=b1_sb[:, 0:1], scale=1.0)
                    # reduce all segments completed so far in this window
                    # (smooths DVE load instead of one big end-of-window op)
                    s_done = b // per          # global seg idx fully written
                    s_prev = max(a // per, w * K_w)
                    if s_done > s_prev:
                        o = s_prev - w * K_w   # seg offset within window
                        n = s_done - s_prev
                        nc.vector.reduce_sum(
                            pooled[:, s_prev:s_prev + n],
                            rw_tiles[w][:, o * per:(o + n) * per].rearrange(
                                "p (s j) -> p s j", j=per),
                            axis=mybir.AxisListType.X)
                    if b == we:
                        del rw_tiles[w]
            else:
                for (off, ln, s, slot) in pieces[gi]:
                    scr = sc_pool.tile([128, GROUP], f32, tag="scr")
                    tgt = (pooled[:, s:s + 1] if slot is None
                           else bsc[:, slot:slot + 1])
                    nc.scalar.activation(
                        scr[:, 0:ln], featT[:, off:off + ln],
                        mybir.ActivationFunctionType.Relu,
                        bias=b1_sb[:, 0:1], scale=1.0,
                        accum_out=tgt)

        # combine split-segment partial sums (general path only)
        for s, slots in ({} if uniform else bslot).items():
            nc.vector.tensor_add(
                pooled[:, s:s + 1], bsc[:, slots[0]:slots[0] + 1],
                bsc[:, slots[1]:slots[1] + 1])
            for extra in slots[2:]:
                nc.vector.tensor_add(
                    pooled[:, s:s + 1], pooled[:, s:s + 1],
                    bsc[:, extra:extra + 1])

        # ---- logits = w2 . pooled (+b2), mask, softmax over types ----
        smf = const_pool.tile([1, n_seg], f32)
        for o in range(0, n_seg, 512):
            n = min(512, n_seg - o)
            lg = ft_pool.tile([1, n], f32, tag="ft")
            nc.tensor.matmul(lg[:], w2_sb[:], pooled[:, o:o + n],
                             start=True, stop=True)
            nc.scalar.activation(
                smf[:, o:o + n], lg[:],
                mybir.ActivationFunctionType.Identity,
                bias=bvec_sb[0:1, 0:1], scale=1.0)

        smg = const_pool.tile([g_core, NUM_TYPE], f32)
        nc.sync.dma_start(smg[:], smf[:])
        # masked = logits * mmul + madd   (mmul in {1,0}, madd in {0,-1e9})
        msk = const_pool.tile([g_core, NUM_TYPE], f32)
        nc.vector.tensor_mul(msk[:], smg[:], mmul_sb[:])
        nc.vector.tensor_add(msk[:], msk[:], madd_sb[:])

        mx = const_pool.tile([g_core, 1], f32)
        nc.vector.reduce_max(mx[:], msk[:], axis=mybir.AxisListType.X)
        nmx = const_pool.tile([g_core, 1], f32)
        nc.vector.tensor_scalar_mul(nmx[:], mx[:], -1.0)
        ex = const_pool.tile([g_core, NUM_TYPE], f32)
        sume = const_pool.tile([g_core, 1], f32)
        nc.scalar.activation(ex[:], msk[:],
                             mybir.ActivationFunctionType.Exp,
                             bias=nmx[:, 0:1], scale=1.0,
                             accum_out=sume[:])
        rse = const_pool.tile([g_core, 1], f32)
        nc.vector.reciprocal(rse[:], sume[:])
        pr = const_pool.tile([g_core, NUM_TYPE], f32)
        nc.vector.tensor_scalar_mul(pr[:], ex[:], rse[:, 0:1])
        nc.sync.dma_start(y, pr[:])

    nc.compile()
    return nc


# ---------------------------------------------------------------------------
# host wrapper
# ---------------------------------------------------------------------------

def _numpy_reference(bond_types_batch, type_count_batch, bond_feat,
                     W1, b1, w2, b2):
    counts = type_count_batch.reshape(-1, NUM_TYPE)
    Bn = counts.shape[0]
    E_ = bond_feat.shape[0]
    feat = np.maximum(bond_feat.reshape(E_, -1) @ W1 + b1, 0.0)
    bonds_per_graph = counts.sum(axis=1)
    cum = np.cumsum(bonds_per_graph)
    graph_id = np.searchsorted(cum, np.arange(E_), side="right")
    seg = graph_id * NUM_TYPE + bond_types_batch
    pooled = np.zeros((Bn * NUM_TYPE, feat.shape[1]), np.float32)
    np.add.at(pooled, seg, feat)
    logits = (pooled @ w2 + b2[0]).reshape(Bn, NUM_TYPE)
    logits = np.where(counts > 0, logits, np.float32(NEG_INF))
    m = logits.max(axis=1, keepdims=True)
    e = np.exp(logits - m)
    return (e / e.sum(axis=1, keepdims=True)).astype(np.float32)


def kernel(**inputs):
    bond_types = np.asarray(inputs["bond_types_batch"])
    counts = np.asarray(inputs["type_count_batch"]).astype(np.int64)
    bond_feat = np.ascontiguousarray(np.asarray(inputs["bond_feat"],
                                                dtype=np.float32))
    W1 = np.asarray(inputs["W1"], dtype=np.float32)
    b1 = np.asarray(inputs["b1"], dtype=np.float32)
    w2 = np.asarray(inputs["w2"], dtype=np.float32)
    b2 = np.asarray(inputs["b2"], dtype=np.float32)

    n_graph = counts.size // NUM_TYPE
    e_total = bond_feat.shape[0]
    bf2d = bond_feat.reshape(e_total, -1)

    ok = (
        n_graph % N_CORES == 0
        and bf2d.shape[1] == FEAT
        and counts.sum() == e_total
        and np.array_equal(
            np.repeat(np.tile(np.arange(NUM_TYPE), n_graph), counts),
            bond_types)
    )
    if ok:
        # identical per-core structure required for a single SPMD program
        per_core = counts.reshape(N_CORES, -1)
        ok = bool((per_core == per_core[0]).all())
    if not ok:
        return _numpy_reference(bond_types, counts.astype(np.int32),
                                bond_feat, W1, b1, w2, b2)

    g_core = n_graph // N_CORES
    seg_counts = per_core[0]
    e_core = int(seg_counts.sum())

    key = (e_core, g_core, seg_counts.tobytes(), MM_MODE, TR_MODE)
    if key not in _CACHE:
        _CACHE[key] = _build_program(e_core, g_core, seg_counts)
    nc = _CACHE[key]

    counts2d = counts.reshape(n_graph, NUM_TYPE)
    mmul_full = (counts2d > 0).astype(np.float32)
    madd_full = np.where(counts2d > 0, 0.0, NEG_INF).astype(np.float32)
    bvec = np.array([b2[0], 0.0], np.float32)
    ident_np = np.eye(128, dtype=np.float32)

    in_maps = []
    for c in range(N_CORES):
        in_maps.append({
            "bf": bf2d[c * e_core:(c + 1) * e_core],
            "w1": W1, "b1": b1, "w2": w2, "bvec": bvec,
            "idin": ident_np,
            "mmul": mmul_full[c * g_core:(c + 1) * g_core],
            "madd": madd_full[c * g_core:(c + 1) * g_core],
        })
    res = run_bass_kernel_spmd(nc, in_maps, core_ids=list(range(N_CORES)))
    return np.concatenate([res.results[c]["y"] for c in range(N_CORES)], axis=0)
